# revision 13
# baseline (speedup 1.0000x reference)
"""Trainium2 Bass kernel for nn_MixtureOfExperts (B=524288, IN=59, E=4, H=64).

Strategy (pure data parallel over 8 cores, 65536 rows each):
 - Host folds BN into weights, collapses the expert head w3@wp -> wep, folds
   |wep| into w2's columns (so stage-3 reduction weights become exact +-1
   signs), and pre-transposes x into feature-major [64, BC] bf16 with a ones
   row at feature 59 (used to inject biases via accumulating matmuls).
 - Everything on-chip is bf16 matmul + f32 psum.  Per 512-row tile:
     stage1: 2 weights-stationary matmuls (experts 01 / 23), N=512
     gate hidden: 1 matmul into a 32-partition strip of a shared psum
     stage2: 2 block-diagonal K=128 matmuls, N=512
     stage3 preds / gate logits / biases: DATA-stationary matmuls - the
       activations (feature-major, in SBUF after relu) are the stationary
       operand and the tiny reduction weights stream, so each costs only
       N=2..8 moving columns.  Outputs land batch-major in one shared psum
       "tail" bank per 8192 rows: chunk cc -> cols [8cc:8cc+8] hold
       [p0 p1 p2 p3 l0 l1 l2 l3] for 128 rows.
 - Relu passes (psum->SBUF bf16 with per-partition bias) are spread across
   the Act, DVE and GPSIMD engines.
 - Tail per 8192 rows: spill bank to SBUF, exp(logits), w = p*exp, row
   reductions over the 4 experts, reciprocal, final product, DMA out.
"""

import numpy as np
import ml_dtypes

import concourse.bass as bass
import concourse.mybir as mybir
import concourse.tile as tile
from concourse import bacc
from concourse.bass_utils import run_bass_kernel_spmd

F32 = mybir.dt.float32
BF16 = mybir.dt.bfloat16
AF = mybir.ActivationFunctionType
ALU = mybir.AluOpType
AX = mybir.AxisListType

B, IN, E, H, EMB, GH = 524288, 59, 4, 64, 32, 32
EPS = 1e-5
NCORES = 8
BC = B // NCORES            # 65536 rows per core
NR = 8                      # rounds per core
RS = BC // NR               # 8192 rows per round
NT = RS // 512              # 16 tiles of 512 per round

# wb (bf16) column layout
W1A0, W1B0, GW10, W2A0, W2B0 = 0, 128, 256, 288, 416
SGA0, SGB0, GW2R0, BEP0 = 544, 546, 548, 552
WB_W = 560
# wf (f32) column layout: c1a c1b c2a c2b gb1t
WF_W = 8

_CACHE = {}

# relu engine assignment: per 16-tile round there are 68 psum->SBUF relu
# passes (64 tile + 4 gate).  GPSIMD cannot touch PSUM (BIR verifier), so
# they split across Act/DVE; Act is slightly faster per pass but also runs
# the exp, DVE runs the reductions/reciprocal.
def _relu_engines(total=36, quota=None):
    quota = quota or {"act": 6, "dve": 30}
    order = []
    frac = {k: 0.0 for k in quota}
    for _ in range(total):
        for k in frac:
            frac[k] += quota[k] / total
        pick = max(frac, key=lambda k: frac[k])
        frac[pick] -= 1.0
        order.append(pick)
    return order

RELU_ENG = _relu_engines()


def _build():
    nc = bacc.Bacc(trn_type="TRN2")
    x_d = nc.dram_tensor("x", (64, BC), BF16, kind="ExternalInput")
    wb_d = nc.dram_tensor("wb", (128, WB_W), BF16, kind="ExternalInput")
    wf_d = nc.dram_tensor("wf", (128, WF_W), F32, kind="ExternalInput")
    out_d = nc.dram_tensor("out", (NR, 128, 64), F32, kind="ExternalOutput")

    with tile.TileContext(nc) as tc:
        with (
            tc.tile_pool(name="consts", bufs=1) as consts,
            tc.tile_pool(name="xp", bufs=2) as xp,
            tc.tile_pool(name="h1p", bufs=2) as h1p,
            tc.tile_pool(name="h2p", bufs=2) as h2p,
            tc.tile_pool(name="g1p", bufs=2) as g1p,
            tc.tile_pool(name="tp", bufs=2) as tp,
            tc.tile_pool(name="p1", bufs=2, space="PSUM") as p1p,
            tc.tile_pool(name="p2a", bufs=1, space="PSUM") as p2ap,
            tc.tile_pool(name="p2b", bufs=1, space="PSUM") as p2bp,
            tc.tile_pool(name="pga", bufs=1, space="PSUM") as pgap,
            tc.tile_pool(name="ptl", bufs=1, space="PSUM") as ptlp,
        ):
            wb = consts.tile([128, WB_W], BF16)
            nc.sync.dma_start(out=wb, in_=wb_d[:, :])
            wf = consts.tile([128, WF_W], F32)
            nc.sync.dma_start(out=wf, in_=wf_d[:, :])

            w1a = wb[0:64, W1A0:W1A0 + 128]
            w1b = wb[0:64, W1B0:W1B0 + 128]
            gw1 = wb[0:64, GW10:GW10 + 32]
            w2a = wb[:, W2A0:W2A0 + 128]
            w2b = wb[:, W2B0:W2B0 + 128]
            sga = wb[:, SGA0:SGA0 + 2]
            sgb = wb[:, SGB0:SGB0 + 2]
            gw2r = wb[:, GW2R0:GW2R0 + 4]
            bep8 = wb[0:64, BEP0:BEP0 + 8]
            c2a = wf[:, 2:3]
            c2b = wf[:, 3:4]

            def relu(eng, out_sb, in_ps, bias_ap):
                if eng == "act":
                    nc.scalar.activation(
                        out_sb, in_ps, AF.Relu,
                        bias=bias_ap if bias_ap is not None else 0.0)
                elif bias_ap is None:
                    nc.vector.tensor_scalar(
                        out_sb, in_ps, 0.0, None, ALU.max)
                else:
                    nc.vector.tensor_scalar(
                        out_sb, in_ps, bias_ap, 0.0, ALU.add, ALU.max)

            for r in range(NR):
                x_sb = xp.tile([64, RS], BF16, tag="x")
                for ch in range(4):
                    cw = RS // 4
                    nc.sync.dma_start(
                        out=x_sb[:, ch * cw:(ch + 1) * cw],
                        in_=x_d[:, r * RS + ch * cw: r * RS + (ch + 1) * cw])

                tail = ptlp.tile([128, 512], F32, tag="tail")
                ri = 0  # relu slot index within round

                def gate_block(g, first):
                    ga = pgap.tile([128, 512], F32, tag="ga")
                    g1r = g1p.tile([128, 512], BF16, tag="g1r")
                    for i in range(4):
                        t = 4 * g + i
                        xs = x_sb[:, 512 * t:512 * t + 512]
                        nc.tensor.matmul(
                            out=ga[32 * i:32 * i + 32, :], lhsT=gw1, rhs=xs,
                            start=True, stop=True, skip_group_check=True,
                            tile_position=(0, 32 * i))
                    nonlocal ri
                    relu(RELU_ENG[ri], g1r, ga, None); ri += 1
                    for i in range(4):
                        t = 4 * g + i
                        for q in range(4):
                            col = 8 * (4 * t + q)
                            xq = x_sb[:, 512 * t + 128 * q:
                                      512 * t + 128 * q + 128]
                            # bias row (bep, gb2): fresh write of the chunk's
                            # 8 cols; logits/preds accumulate on top.
                            nc.tensor.matmul(
                                out=tail[:, col:col + 8],
                                lhsT=xq, rhs=bep8,
                                start=(first and i == 0 and q == 0),
                                stop=False,
                                skip_group_check=True)
                            nc.tensor.matmul(
                                out=tail[:, col + 4:col + 8],
                                lhsT=g1r[32 * i:32 * i + 32,
                                         128 * q:128 * q + 128],
                                rhs=gw2r[32 * i:32 * i + 32, :],
                                start=False, stop=False,
                                skip_group_check=True,
                                tile_position=(32 * i, 0))

                def expert_tile(t):
                    nonlocal ri
                    c0 = 512 * t
                    xs = x_sb[:, c0:c0 + 512]

                    p1 = p1p.tile([128, 1024], F32, tag="p1")
                    nc.tensor.matmul(
                        out=p1[:, 0:512], lhsT=w1a, rhs=xs,
                        start=True, stop=True, skip_group_check=True)
                    nc.tensor.matmul(
                        out=p1[:, 512:1024], lhsT=w1b, rhs=xs,
                        start=True, stop=True, skip_group_check=True)

                    h1r = h1p.tile([128, 1024], BF16, tag="h1r")
                    nc.scalar.activation(h1r, p1, AF.Relu)

                    p2a = p2ap.tile([128, 512], F32, tag="p2a")
                    nc.tensor.matmul(
                        out=p2a, lhsT=w2a, rhs=h1r[:, 0:512],
                        start=True, stop=True)
                    p2b = p2bp.tile([128, 512], F32, tag="p2b")
                    nc.tensor.matmul(
                        out=p2b, lhsT=w2b, rhs=h1r[:, 512:1024],
                        start=True, stop=True)

                    h2ra = h2p.tile([128, 512], BF16, tag="h2ra")
                    relu(RELU_ENG[ri], h2ra, p2a, c2a); ri += 1
                    h2rb = h2p.tile([128, 512], BF16, tag="h2rb")
                    relu(RELU_ENG[ri], h2rb, p2b, c2b); ri += 1

                    for q in range(4):
                        col = 8 * (4 * t + q)
                        nc.tensor.matmul(
                            out=tail[:, col:col + 2],
                            lhsT=h2ra[:, 128 * q:128 * q + 128],
                            rhs=sga,
                            start=False, stop=False,
                            skip_group_check=True)
                        nc.tensor.matmul(
                            out=tail[:, col + 2:col + 4],
                            lhsT=h2rb[:, 128 * q:128 * q + 128],
                            rhs=sgb,
                            start=False,
                            stop=(t == NT - 1 and q == 3),
                            skip_group_check=True)

                # lagged interleave: gate(g) emitted one group ahead of the
                # expert tiles, so logits are long done by round end while
                # Act/DVE stay fed from the previous expert group.
                gate_block(0, True)
                for g in range(4):
                    if g < 3:
                        gate_block(g + 1, False)
                    for i in range(4):
                        expert_tile(4 * g + i)

                # ---- round tail: spill and combine (batch-major)
                tv = tail.rearrange("p (c k) -> p c k", k=8)
                expl = tp.tile([128, 256], F32, tag="expl")
                nc.scalar.activation(
                    expl.rearrange("p (c k) -> p c k", k=4),
                    tv[:, :, 4:8], AF.Exp)
                w_sb = tp.tile([128, 256], F32, tag="w")
                nc.vector.tensor_mul(
                    w_sb.rearrange("p (c k) -> p c k", k=4),
                    tv[:, :, 0:4],
                    expl.rearrange("p (c k) -> p c k", k=4))
                num = tp.tile([128, 64], F32, tag="num")
                nc.vector.tensor_reduce(
                    num, w_sb.rearrange("p (c k) -> p c k", k=4),
                    AX.X, ALU.add)
                den = tp.tile([128, 64], F32, tag="den")
                nc.vector.tensor_reduce(
                    den, expl.rearrange("p (c k) -> p c k", k=4),
                    AX.X, ALU.add)
                rec = tp.tile([128, 64], F32, tag="rec")
                nc.vector.reciprocal(rec, den)
                o_sb = tp.tile([128, 64], F32, tag="o")
                nc.gpsimd.tensor_mul(o_sb, num, rec)
                nc.sync.dma_start(out=out_d[r], in_=o_sb)

    if not nc.is_finalized():
        nc.finalize()
    return nc


def _pack_host(w1, b1, bn1_g, bn1_b, bn1_m, bn1_v, w2, b2, bn2_g, bn2_b,
               bn2_m, bn2_v, w3, b3, wp, bp, gw1, gb1, gw2, gb2):
    f = np.float32
    s1 = (bn1_g / np.sqrt(bn1_v + EPS)).astype(f)
    w1e = (w1 * s1[:, None, :]).astype(f)                       # (E,IN,H)
    c1 = ((b1 - bn1_m) * s1 + bn1_b).astype(f)                  # (E,H)
    s2 = (bn2_g / np.sqrt(bn2_v + EPS)).astype(f)
    w2e = (w2 * s2[:, None, :]).astype(f)                       # (E,H,H)
    c2 = ((b2 - bn2_m) * s2 + bn2_b).astype(f)                  # (E,H)
    wep = np.einsum("ehm,em->eh", w3, wp).astype(f)             # (E,H)
    bep = (np.einsum("em,em->e", b3, wp) + bp).astype(f)        # (E,)

    aw = np.abs(wep)                                            # (E,H)
    sg = np.sign(wep).astype(f)
    w2f = w2e * aw[:, None, :]                                  # cols scaled
    c2f = c2 * aw

    wb = np.zeros((128, WB_W), f)
    wb[0:IN, W1A0:W1A0 + 64] = w1e[0]
    wb[0:IN, W1A0 + 64:W1A0 + 128] = w1e[1]
    wb[0:IN, W1B0:W1B0 + 64] = w1e[2]
    wb[0:IN, W1B0 + 64:W1B0 + 128] = w1e[3]
    wb[IN, W1A0:W1A0 + 128] = np.concatenate([c1[0], c1[1]])
    wb[IN, W1B0:W1B0 + 128] = np.concatenate([c1[2], c1[3]])
    wb[0:IN, GW10:GW10 + 32] = gw1
    wb[IN, GW10:GW10 + 32] = gb1
    wb[0:64, W2A0:W2A0 + 64] = w2f[0]
    wb[64:128, W2A0 + 64:W2A0 + 128] = w2f[1]
    wb[0:64, W2B0:W2B0 + 64] = w2f[2]
    wb[64:128, W2B0 + 64:W2B0 + 128] = w2f[3]
    wb[0:64, SGA0] = sg[0]
    wb[64:128, SGA0 + 1] = sg[1]
    wb[0:64, SGB0] = sg[2]
    wb[64:128, SGB0 + 1] = sg[3]
    for gi in range(4):
        wb[32 * gi:32 * gi + 32, GW2R0:GW2R0 + 4] = gw2
    wb[IN, BEP0:BEP0 + 4] = bep
    wb[IN, BEP0 + 4:BEP0 + 8] = gb2

    wf = np.zeros((128, WF_W), f)
    wf[:, 2] = np.concatenate([c2f[0], c2f[1]])
    wf[:, 3] = np.concatenate([c2f[2], c2f[3]])
    return dict(wb=wb.astype(ml_dtypes.bfloat16), wf=wf)


def _x_core(xc):
    """(BC, 59) f32 -> (64, BC) bf16 feature-major with ones row at 59."""
    xt = np.zeros((64, BC), np.float32)
    xt[:IN] = xc.T
    xt[IN] = 1.0
    return np.ascontiguousarray(xt).astype(ml_dtypes.bfloat16)


def _unpack_out(o):
    """(NR, 128, 64) -> (BC,): row = 8192 r + 512 t + 128 q + b, col=4t+q."""
    o = np.asarray(o, np.float32).reshape(NR, 128, NT, 4)
    return np.ascontiguousarray(o.transpose(0, 2, 3, 1)).reshape(BC)


def _sim_inputs(x_full, packed):
    m = {"x": _x_core(np.asarray(x_full, np.float32)[:BC])}
    m.update(packed)
    return m


def kernel(**inputs):
    x = np.asarray(inputs["x"], dtype=np.float32)
    wk = {k: np.asarray(v, dtype=np.float32) for k, v in inputs.items()
          if k != "x"}
    packed = _pack_host(**wk)

    if "nc" not in _CACHE:
        _CACHE["nc"] = _build()
    nc = _CACHE["nc"]

    in_maps = []
    for c in range(NCORES):
        m = {"x": _x_core(x[c * BC:(c + 1) * BC])}
        m.update(packed)
        in_maps.append(m)

    res = run_bass_kernel_spmd(nc, in_maps, core_ids=list(range(NCORES)))
    _CACHE["last"] = res
    outs = [_unpack_out(r["out"]) for r in res.results]
    return np.concatenate(outs).reshape(B, 1).astype(np.float32)


# revision 22
# speedup vs baseline: 1.0416x; 1.0416x over previous
"""Trainium2 Bass kernel for nn_MixtureOfExperts (B=524288, IN=59, E=4, H=64).

Strategy (pure data parallel over 8 cores, 65536 rows each):
 - Host folds BN into weights, collapses the expert head w3@wp -> wep, folds
   |wep| into w2's columns (so stage-3 reduction weights become exact +-1
   signs), and pre-transposes x into feature-major [64, BC] bf16 with a ones
   row at feature 59 (used to inject biases via accumulating matmuls).
 - Everything on-chip is bf16 matmul + f32 psum.  Per 512-row tile:
     stage1: 2 weights-stationary matmuls (experts 01 / 23), N=512
     gate hidden: 1 matmul into a 32-partition strip of a shared psum
     stage2: 2 block-diagonal K=128 matmuls, N=512
     stage3 preds / gate logits / biases: DATA-stationary matmuls - the
       activations (feature-major, in SBUF after relu) are the stationary
       operand and the tiny reduction weights stream, so each costs only
       N=2..8 moving columns.  Outputs land batch-major in one shared psum
       "tail" bank per 8192 rows: chunk cc -> cols [8cc:8cc+8] hold
       [p0 p1 p2 p3 l0 l1 l2 l3] for 128 rows.
 - Relu passes (psum->SBUF bf16 with per-partition bias) are spread across
   the Act, DVE and GPSIMD engines.
 - Tail per 8192 rows: spill bank to SBUF, exp(logits), w = p*exp, row
   reductions over the 4 experts, reciprocal, final product, DMA out.
"""

import numpy as np
import ml_dtypes

import concourse.bass as bass
import concourse.mybir as mybir
import concourse.tile as tile
from concourse import bacc
from concourse.bass_utils import run_bass_kernel_spmd

F32 = mybir.dt.float32
BF16 = mybir.dt.bfloat16
AF = mybir.ActivationFunctionType
ALU = mybir.AluOpType
AX = mybir.AxisListType

B, IN, E, H, EMB, GH = 524288, 59, 4, 64, 32, 32
EPS = 1e-5
NCORES = 8
BC = B // NCORES            # 65536 rows per core
NR = 8                      # rounds per core
RS = BC // NR               # 8192 rows per round
NT = RS // 512              # 16 tiles of 512 per round

# wb (bf16) column layout
W1A0, W1B0, GW10, W2A0, W2B0 = 0, 128, 256, 288, 416
SGA0, SGB0, GW2R0, BEP0 = 544, 546, 548, 552
WB_W = 560
# wf (f32) column layout: c1a c1b c2a c2b gb1t
WF_W = 8

_CACHE = {}

# relu engine assignment: per 16-tile round there are 68 psum->SBUF relu
# passes (64 tile + 4 gate).  GPSIMD cannot touch PSUM (BIR verifier), so
# they split across Act/DVE; Act is slightly faster per pass but also runs
# the exp, DVE runs the reductions/reciprocal.
def _relu_engines(total=36, quota=None):
    quota = quota or {"act": 6, "dve": 30}
    order = []
    frac = {k: 0.0 for k in quota}
    for _ in range(total):
        for k in frac:
            frac[k] += quota[k] / total
        pick = max(frac, key=lambda k: frac[k])
        frac[pick] -= 1.0
        order.append(pick)
    return order

RELU_ENG = _relu_engines()


def _build():
    nc = bacc.Bacc(trn_type="TRN2")
    x_d = nc.dram_tensor("x", (64, BC), BF16, kind="ExternalInput")
    wb_d = nc.dram_tensor("wb", (128, WB_W), BF16, kind="ExternalInput")
    wf_d = nc.dram_tensor("wf", (128, WF_W), F32, kind="ExternalInput")
    out_d = nc.dram_tensor("out", (NR, 128, 64), F32, kind="ExternalOutput")

    with tile.TileContext(nc) as tc:
        with (
            tc.tile_pool(name="consts", bufs=1) as consts,
            tc.tile_pool(name="xp", bufs=2) as xp,
            tc.tile_pool(name="h1p", bufs=4) as h1p,
            tc.tile_pool(name="h2p", bufs=6) as h2p,
            tc.tile_pool(name="g1p", bufs=3) as g1p,
            tc.tile_pool(name="tp", bufs=2) as tp,
            tc.tile_pool(name="p1", bufs=2, space="PSUM") as p1p,
            tc.tile_pool(name="p2a", bufs=1, space="PSUM") as p2ap,
            tc.tile_pool(name="p2b", bufs=1, space="PSUM") as p2bp,
            tc.tile_pool(name="pga", bufs=1, space="PSUM") as pgap,
            tc.tile_pool(name="ptl", bufs=1, space="PSUM") as ptlp,
        ):
            x0_sb = xp.tile([64, RS], BF16, tag="x")
            cw0 = RS // 4
            nc.sync.dma_start(out=x0_sb[:, 0:cw0], in_=x_d[:, 0:cw0])
            wb = consts.tile([128, WB_W], BF16)
            nc.sync.dma_start(out=wb, in_=wb_d[:, :])
            wf = consts.tile([128, WF_W], F32)
            nc.sync.dma_start(out=wf, in_=wf_d[:, :])

            w1a = wb[0:64, W1A0:W1A0 + 128]
            w1b = wb[0:64, W1B0:W1B0 + 128]
            gw1 = wb[0:64, GW10:GW10 + 32]
            w2a = wb[:, W2A0:W2A0 + 128]
            w2b = wb[:, W2B0:W2B0 + 128]
            sga = wb[:, SGA0:SGA0 + 2]
            sgb = wb[:, SGB0:SGB0 + 2]
            gw2r = wb[:, GW2R0:GW2R0 + 4]
            bep8 = wb[0:64, BEP0:BEP0 + 8]
            c2a = wf[:, 2:3]
            c2b = wf[:, 3:4]

            def relu(eng, out_sb, in_ps, bias_ap):
                if eng == "act":
                    nc.scalar.activation(
                        out_sb, in_ps, AF.Relu,
                        bias=bias_ap if bias_ap is not None else 0.0)
                elif bias_ap is None:
                    nc.vector.tensor_scalar(
                        out_sb, in_ps, 0.0, None, ALU.max)
                else:
                    nc.vector.tensor_scalar(
                        out_sb, in_ps, bias_ap, 0.0, ALU.add, ALU.max)

            for r in range(NR):
                if r == 0:
                    x_sb = x0_sb
                else:
                    x_sb = xp.tile([64, RS], BF16, tag="x")
                cw = RS // 4
                for ch in range(4):
                    if r == 0 and ch == 0:
                        continue
                    nc.sync.dma_start(
                        out=x_sb[:, ch * cw:(ch + 1) * cw],
                        in_=x_d[:, r * RS + ch * cw: r * RS + (ch + 1) * cw])

                tail = ptlp.tile([128, 512], F32, tag="tail")
                ri = 0  # relu slot index within round

                def gate_block(g, first):
                    ga = pgap.tile([128, 512], F32, tag="ga")
                    g1r = g1p.tile([128, 512], BF16, tag="g1r")
                    for i in range(4):
                        t = 4 * g + i
                        xs = x_sb[:, 512 * t:512 * t + 512]
                        nc.tensor.matmul(
                            out=ga[32 * i:32 * i + 32, :], lhsT=gw1, rhs=xs,
                            start=True, stop=True, skip_group_check=True,
                            tile_position=(0, 32 * i))
                    nonlocal ri
                    relu(RELU_ENG[ri], g1r, ga, None); ri += 1
                    for i in range(4):
                        t = 4 * g + i
                        for q in range(4):
                            col = 8 * (4 * t + q)
                            xq = x_sb[:, 512 * t + 128 * q:
                                      512 * t + 128 * q + 128]
                            # bias row (bep, gb2): fresh write of the chunk's
                            # 8 cols; logits/preds accumulate on top.
                            nc.tensor.matmul(
                                out=tail[:, col:col + 8],
                                lhsT=xq, rhs=bep8,
                                start=(first and i == 0 and q == 0),
                                stop=False,
                                skip_group_check=True)
                            nc.tensor.matmul(
                                out=tail[:, col + 4:col + 8],
                                lhsT=g1r[32 * i:32 * i + 32,
                                         128 * q:128 * q + 128],
                                rhs=gw2r[32 * i:32 * i + 32, :],
                                start=False, stop=False,
                                skip_group_check=True,
                                tile_position=(32 * i, 0))

                def expert_tile(t):
                    nonlocal ri
                    c0 = 512 * t
                    xs = x_sb[:, c0:c0 + 512]

                    p1 = p1p.tile([128, 1024], F32, tag="p1")
                    nc.tensor.matmul(
                        out=p1[:, 0:512], lhsT=w1a, rhs=xs,
                        start=True, stop=True, skip_group_check=True)
                    nc.tensor.matmul(
                        out=p1[:, 512:1024], lhsT=w1b, rhs=xs,
                        start=True, stop=True, skip_group_check=True)

                    h1r = h1p.tile([128, 1024], BF16, tag="h1r")
                    nc.scalar.activation(h1r, p1, AF.Relu)

                    p2a = p2ap.tile([128, 512], F32, tag="p2a")
                    nc.tensor.matmul(
                        out=p2a, lhsT=w2a, rhs=h1r[:, 0:512],
                        start=True, stop=True)
                    p2b = p2bp.tile([128, 512], F32, tag="p2b")
                    nc.tensor.matmul(
                        out=p2b, lhsT=w2b, rhs=h1r[:, 512:1024],
                        start=True, stop=True)

                    h2ra = h2p.tile([128, 512], BF16, tag="h2ra")
                    relu(RELU_ENG[ri], h2ra, p2a, c2a); ri += 1
                    h2rb = h2p.tile([128, 512], BF16, tag="h2rb")
                    relu(RELU_ENG[ri], h2rb, p2b, c2b); ri += 1
                    for q in range(4):
                        col = 8 * (4 * t + q)
                        nc.tensor.matmul(
                            out=tail[:, col:col + 2],
                            lhsT=h2ra[:, 128 * q:128 * q + 128],
                            rhs=sga,
                            start=False, stop=False,
                            skip_group_check=True)
                        nc.tensor.matmul(
                            out=tail[:, col + 2:col + 4],
                            lhsT=h2rb[:, 128 * q:128 * q + 128],
                            rhs=sgb,
                            start=False,
                            stop=(t == NT - 1 and q == 3),
                            skip_group_check=True)

                # staggered interleave: gate(g) leads its expert tiles by
                # ~2 tiles so logits/bias are ready early without starving
                # Act/DVE at round starts.
                gate_block(0, True)
                expert_tile(0)
                expert_tile(1)
                gate_block(1, False)
                expert_tile(2)
                expert_tile(3)
                expert_tile(4)
                expert_tile(5)
                gate_block(2, False)
                expert_tile(6)
                expert_tile(7)
                expert_tile(8)
                expert_tile(9)
                gate_block(3, False)
                for t in range(10, 16):
                    expert_tile(t)

                # ---- round tail: combine (batch-major) over chunk range
                def round_tail(lo, hi, suf):
                    n = hi - lo
                    tv = tail.rearrange("p (c k) -> p c k", k=8)[:, lo:hi]
                    expl = tp.tile([128, n * 4], F32, tag="expl" + suf)
                    ev = expl.rearrange("p (c k) -> p c k", k=4)
                    nc.scalar.activation(ev, tv[:, :, 4:8], AF.Exp)
                    w_sb = tp.tile([128, n * 4], F32, tag="w" + suf)
                    wv = w_sb.rearrange("p (c k) -> p c k", k=4)
                    nc.vector.tensor_mul(wv, tv[:, :, 0:4], ev)
                    num = tp.tile([128, n], F32, tag="num" + suf)
                    nc.vector.tensor_reduce(num, wv, AX.X, ALU.add)
                    den = tp.tile([128, n], F32, tag="den" + suf)
                    nc.vector.tensor_reduce(den, ev, AX.X, ALU.add)
                    rec = tp.tile([128, n], F32, tag="rec" + suf)
                    nc.vector.reciprocal(rec, den)
                    o_sb = tp.tile([128, n], F32, tag="o" + suf)
                    nc.gpsimd.tensor_mul(o_sb, num, rec)
                    nc.sync.dma_start(out=out_d[r][:, lo:hi], in_=o_sb)

                if r == NR - 1:
                    round_tail(0, 32, "a")
                    round_tail(32, 64, "b")
                else:
                    round_tail(0, 64, "")

    if not nc.is_finalized():
        nc.finalize()
    return nc


def _pack_host(w1, b1, bn1_g, bn1_b, bn1_m, bn1_v, w2, b2, bn2_g, bn2_b,
               bn2_m, bn2_v, w3, b3, wp, bp, gw1, gb1, gw2, gb2):
    f = np.float32
    s1 = (bn1_g / np.sqrt(bn1_v + EPS)).astype(f)
    w1e = (w1 * s1[:, None, :]).astype(f)                       # (E,IN,H)
    c1 = ((b1 - bn1_m) * s1 + bn1_b).astype(f)                  # (E,H)
    s2 = (bn2_g / np.sqrt(bn2_v + EPS)).astype(f)
    w2e = (w2 * s2[:, None, :]).astype(f)                       # (E,H,H)
    c2 = ((b2 - bn2_m) * s2 + bn2_b).astype(f)                  # (E,H)
    wep = np.einsum("ehm,em->eh", w3, wp).astype(f)             # (E,H)
    bep = (np.einsum("em,em->e", b3, wp) + bp).astype(f)        # (E,)

    aw = np.abs(wep)                                            # (E,H)
    sg = np.sign(wep).astype(f)
    w2f = w2e * aw[:, None, :]                                  # cols scaled
    c2f = c2 * aw

    wb = np.zeros((128, WB_W), f)
    wb[0:IN, W1A0:W1A0 + 64] = w1e[0]
    wb[0:IN, W1A0 + 64:W1A0 + 128] = w1e[1]
    wb[0:IN, W1B0:W1B0 + 64] = w1e[2]
    wb[0:IN, W1B0 + 64:W1B0 + 128] = w1e[3]
    wb[IN, W1A0:W1A0 + 128] = np.concatenate([c1[0], c1[1]])
    wb[IN, W1B0:W1B0 + 128] = np.concatenate([c1[2], c1[3]])
    wb[0:IN, GW10:GW10 + 32] = gw1
    wb[IN, GW10:GW10 + 32] = gb1
    wb[0:64, W2A0:W2A0 + 64] = w2f[0]
    wb[64:128, W2A0 + 64:W2A0 + 128] = w2f[1]
    wb[0:64, W2B0:W2B0 + 64] = w2f[2]
    wb[64:128, W2B0 + 64:W2B0 + 128] = w2f[3]
    wb[0:64, SGA0] = sg[0]
    wb[64:128, SGA0 + 1] = sg[1]
    wb[0:64, SGB0] = sg[2]
    wb[64:128, SGB0 + 1] = sg[3]
    for gi in range(4):
        wb[32 * gi:32 * gi + 32, GW2R0:GW2R0 + 4] = gw2
    wb[IN, BEP0:BEP0 + 4] = bep
    wb[IN, BEP0 + 4:BEP0 + 8] = gb2

    wf = np.zeros((128, WF_W), f)
    wf[:, 2] = np.concatenate([c2f[0], c2f[1]])
    wf[:, 3] = np.concatenate([c2f[2], c2f[3]])
    return dict(wb=wb.astype(ml_dtypes.bfloat16), wf=wf)


def _x_core(xc):
    """(BC, 59) f32 -> (64, BC) bf16 feature-major with ones row at 59."""
    xt = np.zeros((64, BC), np.float32)
    xt[:IN] = xc.T
    xt[IN] = 1.0
    return np.ascontiguousarray(xt).astype(ml_dtypes.bfloat16)


def _unpack_out(o):
    """(NR, 128, 64) -> (BC,): row = 8192 r + 512 t + 128 q + b, col=4t+q."""
    o = np.asarray(o, np.float32).reshape(NR, 128, NT, 4)
    return np.ascontiguousarray(o.transpose(0, 2, 3, 1)).reshape(BC)


def _sim_inputs(x_full, packed):
    m = {"x": _x_core(np.asarray(x_full, np.float32)[:BC])}
    m.update(packed)
    return m


def kernel(**inputs):
    x = np.asarray(inputs["x"], dtype=np.float32)
    wk = {k: np.asarray(v, dtype=np.float32) for k, v in inputs.items()
          if k != "x"}
    packed = _pack_host(**wk)

    if "nc" not in _CACHE:
        _CACHE["nc"] = _build()
    nc = _CACHE["nc"]

    in_maps = []
    for c in range(NCORES):
        m = {"x": _x_core(x[c * BC:(c + 1) * BC])}
        m.update(packed)
        in_maps.append(m)

    res = run_bass_kernel_spmd(nc, in_maps, core_ids=list(range(NCORES)))
    _CACHE["last"] = res
    outs = [_unpack_out(r["out"]) for r in res.results]
    return np.concatenate(outs).reshape(B, 1).astype(np.float32)
